# revision 3
# baseline (speedup 1.0000x reference)
"""Memory-efficient multi-head attention block on 8 TRN2 NeuronCores.

Computes (matching torch nn.Linear conventions, W is [out, in]):
    q, k, v = x@Wq.T, x@Wk.T, x@Wv.T          [B, S, H] -> heads [B, NH, S, HD]
    out     = softmax(q k^T / sqrt(HD)) v      per head
    y       = concat_heads(out) @ Wo.T + bo

Sharding: head-parallel tensor parallelism. Each of the 8 cores owns 2 of the
16 heads: Wq/Wk/Wv are sharded on their output dim, Wo on its input dim. Each
core computes a full-shape partial y (its heads' contribution through Wo);
host sums the 8 partials and adds the bias.

Layout trick: everything on device runs transpose-free.  The host feeds
x pre-transposed as xT[b] = x[b].T ([H, S]); then
  - qT/kT (per head [HD, S]) come out of matmuls directly (weights as lhsT),
  - v     (per head [S, HD]) uses xT tiles as lhsT,
  - scores^T [k, q] = kT_tile.T @ qT (contract over HD partitions),
  - attn-out^T [HD, q] = v_tile.T @ exp(scores^T) (contract over k partitions),
  - y tiles [tok, out] = attnT_tile.T @ WoT (contract over HD partitions).
Softmax skips the max-subtraction (scaled scores are ~N(0,1); exp is safe in
fp32) and builds the denominator with a DVE accumulation over k-tiles plus an
all-ones matmul that both finishes the sum across partitions and broadcasts
it; 1/norm is exp(-ln(norm)) on ScalarE (exp and ln share one ACT table set;
the dedicated Reciprocal op is disallowed/slow).

Matmuls run in float32r (TF32-like: full rate at free-dim >= 256, measured
rel. err ~1.5e-4 for a K=2048 contraction vs fp32's 4x slowdown).
"""
import sys

sys.path.insert(0, "/opt/trn_rl_repo")

import numpy as np

import concourse.bass as bass  # noqa: F401  (engine registry import side effects)
import concourse.mybir as mybir
import concourse.tile as tile
from concourse import bacc
from concourse.bass_utils import run_bass_kernel_spmd

B, S, H, NH = 2, 2048, 2048, 16
HD = H // NH            # 128
NCORES = 8
HPC = NH // NCORES      # heads per core = 2
DLOC = HPC * HD         # 256 local head dims per core
SCALE = 1.0 / float(np.sqrt(HD))

F32 = mybir.dt.float32
F32R = mybir.dt.float32r
EXP = mybir.ActivationFunctionType.Exp
LN = mybir.ActivationFunctionType.Ln

KS = H // 128           # 16 contraction subtiles for the projections
NTT = S // 512          # 4 token tiles of 512
NKT = S // 128          # 16 key tiles of 128
NQT = S // 512          # 4 query tiles of 512
NOT_ = H // 512         # 4 output tiles of 512
NTT2 = S // 128         # 16 token tiles of 128 (output projection)


def _build():
    nc = bacc.Bacc("TRN2", target_bir_lowering=False, debug=False, num_devices=NCORES)

    xT_d = nc.dram_tensor("xT", [B, H, S], F32R, kind="ExternalInput").ap()
    ones_d = nc.dram_tensor("ones", [128, 128], F32R, kind="ExternalInput").ap()
    wq_d = nc.dram_tensor("wq", [H, DLOC], F32R, kind="ExternalInput").ap()
    wk_d = nc.dram_tensor("wk", [H, DLOC], F32R, kind="ExternalInput").ap()
    wv_d = nc.dram_tensor("wv", [H, DLOC], F32R, kind="ExternalInput").ap()
    wo_d = nc.dram_tensor("wo", [DLOC, H], F32R, kind="ExternalInput").ap()
    y_d = nc.dram_tensor("y", [B, S, H], F32, kind="ExternalOutput").ap()

    with tile.TileContext(nc) as tc:
        with tc.tile_pool(name="sb", bufs=1) as sb, \
             tc.tile_pool(name="ps", bufs=1, space="PSUM") as ps:

            ones = sb.tile([128, 128], F32R, tag="ones", bufs=1)
            nc.sync.dma_start(ones, ones_d)

            # Resident weights: [128, KS, DLOC] with the contraction dim on
            # partitions; wo as [128, HPC, H].
            wq_s = sb.tile([128, KS, DLOC], F32R, tag="wq", bufs=1)
            wk_s = sb.tile([128, KS, DLOC], F32R, tag="wk", bufs=1)
            wv_s = sb.tile([128, KS, DLOC], F32R, tag="wv", bufs=1)
            wo_s = sb.tile([128, HPC, H], F32R, tag="wo", bufs=1)
            for ks in range(KS):
                nc.sync.dma_start(wq_s[:, ks], wq_d[ks * 128:(ks + 1) * 128, :])
                nc.sync.dma_start(wk_s[:, ks], wk_d[ks * 128:(ks + 1) * 128, :])
                nc.sync.dma_start(wv_s[:, ks], wv_d[ks * 128:(ks + 1) * 128, :])
            for h in range(HPC):
                nc.sync.dma_start(wo_s[:, h], wo_d[h * 128:(h + 1) * 128, :])

            for b in range(B):
                # ---- q/k/v projections (one streaming pass over xT[b]) ----
                qTb = sb.tile([128, HPC, S], F32R, tag="qTb", bufs=1)
                kTb = sb.tile([128, HPC, S], F32R, tag="kTb", bufs=1)
                vb = sb.tile([128, NKT, DLOC], F32R, tag="vb", bufs=1)
                aoTb = sb.tile([128, HPC, S], F32R, tag="aoTb", bufs=1)

                for tt in range(NTT):
                    q_ps = [ps.tile([128, 512], F32, tag="ps", bufs=8, name=f"qps{b}{tt}{m}") for m in range(2)]
                    k_ps = [ps.tile([128, 512], F32, tag="ps", bufs=8, name=f"kps{b}{tt}{m}") for m in range(2)]
                    v_ps = [ps.tile([128, 256], F32, tag="ps", bufs=8, name=f"vps{b}{tt}{m}") for m in range(4)]
                    for ks in range(KS):
                        xs = sb.tile([128, 512], F32R, tag="xs", bufs=4, name=f"xs{b}{tt}{ks}")
                        nc.sync.dma_start(
                            xs, xT_d[b, ks * 128:(ks + 1) * 128, tt * 512:(tt + 1) * 512])
                        st, sp = ks == 0, ks == KS - 1
                        for m in range(2):
                            nc.tensor.matmul(q_ps[m], wq_s[:, ks, m * 128:(m + 1) * 128],
                                             xs, start=st, stop=sp)
                        for m in range(2):
                            nc.tensor.matmul(k_ps[m], wk_s[:, ks, m * 128:(m + 1) * 128],
                                             xs, start=st, stop=sp)
                        for t4 in range(4):
                            nc.tensor.matmul(
                                v_ps[t4],
                                xs[:, t4 * 128:(t4 + 1) * 128], wv_s[:, ks],
                                start=st, stop=sp)
                    for m in range(2):
                        nc.vector.tensor_copy(qTb[:, m, tt * 512:(tt + 1) * 512], q_ps[m])
                        nc.vector.tensor_copy(kTb[:, m, tt * 512:(tt + 1) * 512], k_ps[m])
                    for t4 in range(4):
                        nc.vector.tensor_copy(vb[:, tt * 4 + t4, :], v_ps[t4])

                # ---- attention per local head ----
                for h in range(HPC):
                    unn = [ps.tile([128, 512], F32, tag="ps", bufs=8, name=f"unn{b}{h}{qt}") for qt in range(NQT)]
                    accs = [sb.tile([128, 512], F32R, tag="acc", bufs=8, name=f"acc{b}{h}{qt}") for qt in range(NQT)]
                    for kt in range(NKT):
                        st, sp = kt == 0, kt == NKT - 1
                        for qt in range(NQT):
                            sps = ps.tile([128, 512], F32, tag="ps", bufs=8, name=f"sps{b}{h}{kt}{qt}")
                            nc.tensor.matmul(sps, kTb[:, h, kt * 128:(kt + 1) * 128],
                                             qTb[:, h, qt * 512:(qt + 1) * 512],
                                             start=True, stop=True)
                            e = sb.tile([128, 512], F32R, tag="e", bufs=4, name=f"e{b}{h}{kt}{qt}")
                            nc.scalar.activation(e, sps, EXP, scale=SCALE)
                            nc.tensor.matmul(unn[qt], vb[:, kt, h * 128:(h + 1) * 128],
                                             e, start=st, stop=sp)
                            if kt == 0:
                                nc.vector.tensor_copy(accs[qt], e)
                            else:
                                nc.vector.tensor_add(accs[qt], accs[qt], e)
                    for qt in range(NQT):
                        nps = ps.tile([128, 512], F32, tag="ps", bufs=8, name=f"nps{b}{h}{qt}")
                        nc.tensor.matmul(nps, ones, accs[qt], start=True, stop=True)
                        lnn = sb.tile([128, 512], F32, tag="rc", bufs=4, name=f"ln{b}{h}{qt}")
                        nc.scalar.activation(lnn, nps, LN)
                        rc = sb.tile([128, 512], F32, tag="rc", bufs=4, name=f"rc{b}{h}{qt}")
                        nc.scalar.activation(rc, lnn, EXP, scale=-1.0)
                        nc.vector.tensor_mul(aoTb[:, h, qt * 512:(qt + 1) * 512],
                                             unn[qt], rc)

                # ---- output projection (partial y for this core's heads) ----
                for t2 in range(NTT2):
                    y_ps = [ps.tile([128, 512], F32, tag="ps", bufs=8, name=f"yps{b}{t2}{ot}") for ot in range(NOT_)]
                    for h in range(HPC):
                        for ot in range(NOT_):
                            nc.tensor.matmul(y_ps[ot], aoTb[:, h, t2 * 128:(t2 + 1) * 128],
                                             wo_s[:, h, ot * 512:(ot + 1) * 512],
                                             start=(h == 0), stop=(h == HPC - 1))
                    for ot in range(NOT_):
                        ysb = sb.tile([128, 512], F32, tag="ysb", bufs=4, name=f"ysb{b}{t2}{ot}")
                        nc.vector.tensor_copy(ysb, y_ps[ot])
                        nc.sync.dma_start(
                            y_d[b, t2 * 128:(t2 + 1) * 128, ot * 512:(ot + 1) * 512], ysb)

    nc.compile()
    return nc


_NC = None


def _get_nc():
    global _NC
    if _NC is None:
        _NC = _build()
    return _NC


def kernel(x, Wq, Wk, Wv, Wo, bo):
    x = np.asarray(x, dtype=np.float32)
    Wq = np.asarray(Wq, dtype=np.float32)
    Wk = np.asarray(Wk, dtype=np.float32)
    Wv = np.asarray(Wv, dtype=np.float32)
    Wo = np.asarray(Wo, dtype=np.float32)
    bo = np.asarray(bo, dtype=np.float32)

    nc = _get_nc()
    xT = np.ascontiguousarray(x.transpose(0, 2, 1))
    in_maps = []
    for c in range(NCORES):
        sl = slice(c * DLOC, (c + 1) * DLOC)
        in_maps.append({
            "xT": xT,
            "ones": np.ones((128, 128), dtype=np.float32),
            "wq": np.ascontiguousarray(Wq[sl, :].T),
            "wk": np.ascontiguousarray(Wk[sl, :].T),
            "wv": np.ascontiguousarray(Wv[sl, :].T),
            "wo": np.ascontiguousarray(Wo[:, sl].T),
        })
    res = run_bass_kernel_spmd(nc, in_maps, list(range(NCORES)))
    y = np.zeros((B, S, H), dtype=np.float32)
    for c in range(NCORES):
        y += np.asarray(res.results[c]["y"])
    y += bo
    return y


# revision 4
# speedup vs baseline: 1.0993x; 1.0993x over previous
"""Memory-efficient multi-head attention block on 8 TRN2 NeuronCores.

Computes (matching torch nn.Linear conventions, W is [out, in]):
    q, k, v = x@Wq.T, x@Wk.T, x@Wv.T          [B, S, H] -> heads [B, NH, S, HD]
    out     = softmax(q k^T / sqrt(HD)) v      per head
    y       = concat_heads(out) @ Wo.T + bo

Sharding: head-parallel tensor parallelism. Each of the 8 cores owns 2 of the
16 heads: Wq/Wk/Wv are sharded on their output dim, Wo on its input dim. Each
core computes a full-shape partial y (its heads' contribution through Wo);
host sums the 8 partials and adds the bias.

Layout trick: everything on device runs transpose-free.  The host feeds
x pre-transposed as xT[b] = x[b].T ([H, S]); then
  - qT/kT (per head [HD, S]) come out of matmuls directly (weights as lhsT),
  - v     (per head [S, HD]) uses xT tiles as lhsT,
  - scores^T [k, q] = kT_tile.T @ qT (contract over HD partitions),
  - attn-out^T [HD, q] = v_tile.T @ exp(scores^T) (contract over k partitions),
  - y tiles [tok, out] = attnT_tile.T @ WoT (contract over HD partitions).
Softmax skips the max-subtraction (scaled scores are ~N(0,1); exp is safe in
fp32) and builds the denominator with a DVE accumulation over k-tiles plus an
all-ones matmul that both finishes the sum across partitions and broadcasts
it; 1/norm is exp(-ln(norm)) on ScalarE (exp and ln share one ACT table set;
the dedicated Reciprocal op is disallowed/slow).

Matmuls run in float32r (TF32-like: full rate at free-dim >= 256, measured
rel. err ~1.5e-4 for a K=2048 contraction vs fp32's 4x slowdown).
"""
import sys

sys.path.insert(0, "/opt/trn_rl_repo")

import numpy as np

import concourse.bass as bass  # noqa: F401  (engine registry import side effects)
import concourse.mybir as mybir
import concourse.tile as tile
from concourse import bacc
from concourse.bass_utils import run_bass_kernel_spmd

B, S, H, NH = 2, 2048, 2048, 16
HD = H // NH            # 128
NCORES = 8
HPC = NH // NCORES      # heads per core = 2
DLOC = HPC * HD         # 256 local head dims per core
SCALE = 1.0 / float(np.sqrt(HD))

F32 = mybir.dt.float32
F32R = mybir.dt.float32r
EXP = mybir.ActivationFunctionType.Exp
LN = mybir.ActivationFunctionType.Ln

KS = H // 128           # 16 contraction subtiles for the projections
NTT = S // 512          # 4 token tiles of 512
NKT = S // 128          # 16 key tiles of 128
NQT = S // 512          # 4 query tiles of 512
NOT_ = H // 512         # 4 output tiles of 512
NTT2 = S // 128         # 16 token tiles of 128 (output projection)


def _build():
    nc = bacc.Bacc("TRN2", target_bir_lowering=False, debug=False, num_devices=NCORES)

    xT_d = nc.dram_tensor("xT", [B, H, S], F32R, kind="ExternalInput").ap()
    ones_d = nc.dram_tensor("ones", [128, 128], F32R, kind="ExternalInput").ap()
    wq_d = nc.dram_tensor("wq", [H, DLOC], F32R, kind="ExternalInput").ap()
    wk_d = nc.dram_tensor("wk", [H, DLOC], F32R, kind="ExternalInput").ap()
    wv_d = nc.dram_tensor("wv", [H, DLOC], F32R, kind="ExternalInput").ap()
    wo_d = nc.dram_tensor("wo", [DLOC, H], F32R, kind="ExternalInput").ap()
    y_d = nc.dram_tensor("y", [B, S, H], F32, kind="ExternalOutput").ap()

    with tile.TileContext(nc) as tc:
        with tc.tile_pool(name="sb", bufs=1) as sb, \
             tc.tile_pool(name="ps", bufs=1, space="PSUM") as ps:

            ones = sb.tile([128, 128], F32R, tag="ones", bufs=1)

            # Resident weights: [128, KS, DLOC] with the contraction dim on
            # partitions; wo as [128, HPC, H].
            wq_s = sb.tile([128, KS, DLOC], F32R, tag="wq", bufs=1)
            wk_s = sb.tile([128, KS, DLOC], F32R, tag="wk", bufs=1)
            wv_s = sb.tile([128, KS, DLOC], F32R, tag="wv", bufs=1)
            wo_s = sb.tile([128, HPC, H], F32R, tag="wo", bufs=1)
            for ks in range(KS):
                nc.gpsimd.dma_start(wq_s[:, ks], wq_d[ks * 128:(ks + 1) * 128, :])
                nc.gpsimd.dma_start(wk_s[:, ks], wk_d[ks * 128:(ks + 1) * 128, :])
                nc.gpsimd.dma_start(wv_s[:, ks], wv_d[ks * 128:(ks + 1) * 128, :])
            nc.gpsimd.dma_start(ones, ones_d)
            for h in range(HPC):
                nc.gpsimd.dma_start(wo_s[:, h], wo_d[h * 128:(h + 1) * 128, :])

            for b in range(B):
                # ---- q/k/v projections (one streaming pass over xT[b]) ----
                qTb = sb.tile([128, HPC, S], F32R, tag="qTb", bufs=1)
                kTb = sb.tile([128, HPC, S], F32R, tag="kTb", bufs=1)
                vb = sb.tile([128, NKT, DLOC], F32R, tag="vb", bufs=1)
                aoTb = sb.tile([128, HPC, S], F32R, tag="aoTb", bufs=1)

                for tt in range(NTT):
                    q_ps = [ps.tile([128, 512], F32, tag="ps", bufs=8, name=f"qps{b}{tt}{m}") for m in range(2)]
                    k_ps = [ps.tile([128, 512], F32, tag="ps", bufs=8, name=f"kps{b}{tt}{m}") for m in range(2)]
                    v_ps = [ps.tile([128, 256], F32, tag="ps", bufs=8, name=f"vps{b}{tt}{m}") for m in range(4)]
                    for ks in range(KS):
                        xs = sb.tile([128, 512], F32R, tag="xs", bufs=6, name=f"xs{b}{tt}{ks}")
                        nc.sync.dma_start(
                            xs, xT_d[b, ks * 128:(ks + 1) * 128, tt * 512:(tt + 1) * 512])
                        st, sp = ks == 0, ks == KS - 1
                        for m in range(2):
                            nc.tensor.matmul(q_ps[m], wq_s[:, ks, m * 128:(m + 1) * 128],
                                             xs, start=st, stop=sp)
                        for m in range(2):
                            nc.tensor.matmul(k_ps[m], wk_s[:, ks, m * 128:(m + 1) * 128],
                                             xs, start=st, stop=sp)
                        for t4 in range(4):
                            nc.tensor.matmul(
                                v_ps[t4],
                                xs[:, t4 * 128:(t4 + 1) * 128], wv_s[:, ks],
                                start=st, stop=sp)
                    for m in range(2):
                        nc.vector.tensor_copy(qTb[:, m, tt * 512:(tt + 1) * 512], q_ps[m])
                        nc.vector.tensor_copy(kTb[:, m, tt * 512:(tt + 1) * 512], k_ps[m])
                    for t4 in range(4):
                        nc.vector.tensor_copy(vb[:, tt * 4 + t4, :], v_ps[t4])

                # ---- attention per local head ----
                for h in range(HPC):
                    unn = [ps.tile([128, 512], F32, tag="ps", bufs=8, name=f"unn{b}{h}{qt}") for qt in range(NQT)]
                    accs = [sb.tile([128, 512], F32R, tag="acc", bufs=8, name=f"acc{b}{h}{qt}") for qt in range(NQT)]
                    for kt in range(NKT):
                        st, sp = kt == 0, kt == NKT - 1
                        for qt in range(NQT):
                            sps = ps.tile([128, 512], F32, tag="ps", bufs=8, name=f"sps{b}{h}{kt}{qt}")
                            nc.tensor.matmul(sps, kTb[:, h, kt * 128:(kt + 1) * 128],
                                             qTb[:, h, qt * 512:(qt + 1) * 512],
                                             start=True, stop=True)
                            e = sb.tile([128, 512], F32R, tag="e", bufs=6, name=f"e{b}{h}{kt}{qt}")
                            nc.scalar.activation(e, sps, EXP, scale=SCALE)
                            nc.tensor.matmul(unn[qt], vb[:, kt, h * 128:(h + 1) * 128],
                                             e, start=st, stop=sp)
                            if kt == 0:
                                nc.vector.tensor_copy(accs[qt], e)
                            else:
                                nc.vector.tensor_add(accs[qt], accs[qt], e)
                    for qt in range(NQT):
                        nps = ps.tile([128, 512], F32, tag="ps", bufs=8, name=f"nps{b}{h}{qt}")
                        nc.tensor.matmul(nps, ones, accs[qt], start=True, stop=True)
                        lnn = sb.tile([128, 512], F32, tag="rc", bufs=4, name=f"ln{b}{h}{qt}")
                        nc.scalar.activation(lnn, nps, LN)
                        rc = sb.tile([128, 512], F32, tag="rc", bufs=4, name=f"rc{b}{h}{qt}")
                        nc.scalar.activation(rc, lnn, EXP, scale=-1.0)
                        nc.vector.tensor_mul(aoTb[:, h, qt * 512:(qt + 1) * 512],
                                             unn[qt], rc)

                # ---- output projection (partial y for this core's heads) ----
                for t2 in range(NTT2):
                    y_ps = [ps.tile([128, 512], F32, tag="ps", bufs=8, name=f"yps{b}{t2}{ot}") for ot in range(NOT_)]
                    for h in range(HPC):
                        for ot in range(NOT_):
                            nc.tensor.matmul(y_ps[ot], aoTb[:, h, t2 * 128:(t2 + 1) * 128],
                                             wo_s[:, h, ot * 512:(ot + 1) * 512],
                                             start=(h == 0), stop=(h == HPC - 1))
                    for ot in range(NOT_):
                        ysb = sb.tile([128, 512], F32, tag="ysb", bufs=4, name=f"ysb{b}{t2}{ot}")
                        nc.vector.tensor_copy(ysb, y_ps[ot])
                        nc.sync.dma_start(
                            y_d[b, t2 * 128:(t2 + 1) * 128, ot * 512:(ot + 1) * 512], ysb)

    nc.compile()
    return nc


_NC = None


def _get_nc():
    global _NC
    if _NC is None:
        _NC = _build()
    return _NC


def kernel(x, Wq, Wk, Wv, Wo, bo):
    x = np.asarray(x, dtype=np.float32)
    Wq = np.asarray(Wq, dtype=np.float32)
    Wk = np.asarray(Wk, dtype=np.float32)
    Wv = np.asarray(Wv, dtype=np.float32)
    Wo = np.asarray(Wo, dtype=np.float32)
    bo = np.asarray(bo, dtype=np.float32)

    nc = _get_nc()
    xT = np.ascontiguousarray(x.transpose(0, 2, 1))
    in_maps = []
    for c in range(NCORES):
        sl = slice(c * DLOC, (c + 1) * DLOC)
        in_maps.append({
            "xT": xT,
            "ones": np.ones((128, 128), dtype=np.float32),
            "wq": np.ascontiguousarray(Wq[sl, :].T),
            "wk": np.ascontiguousarray(Wk[sl, :].T),
            "wv": np.ascontiguousarray(Wv[sl, :].T),
            "wo": np.ascontiguousarray(Wo[:, sl].T),
        })
    res = run_bass_kernel_spmd(nc, in_maps, list(range(NCORES)))
    y = np.zeros((B, S, H), dtype=np.float32)
    for c in range(NCORES):
        y += np.asarray(res.results[c]["y"])
    y += bo
    return y


# revision 6
# speedup vs baseline: 1.1125x; 1.0120x over previous
"""Memory-efficient multi-head attention block on 8 TRN2 NeuronCores.

Computes (matching torch nn.Linear conventions, W is [out, in]):
    q, k, v = x@Wq.T, x@Wk.T, x@Wv.T          [B, S, H] -> heads [B, NH, S, HD]
    out     = softmax(q k^T / sqrt(HD)) v      per head
    y       = concat_heads(out) @ Wo.T + bo

Sharding: head-parallel tensor parallelism. Each of the 8 cores owns 2 of the
16 heads: Wq/Wk/Wv are sharded on their output dim, Wo on its input dim. Each
core computes a full-shape partial y (its heads' contribution through Wo);
host sums the 8 partials and adds the bias.

Layout trick: everything on device runs transpose-free.  The host feeds
x pre-transposed as xT[b] = x[b].T ([H, S]); then
  - qT/kT (per head [HD, S]) come out of matmuls directly (weights as lhsT),
  - v     (per head [S, HD]) uses xT tiles as lhsT,
  - scores^T [k, q] = kT_tile.T @ qT (contract over HD partitions),
  - attn-out^T [HD, q] = v_tile.T @ exp(scores^T) (contract over k partitions),
  - y tiles [tok, out] = attnT_tile.T @ WoT (contract over HD partitions).
Softmax skips the max-subtraction (scaled scores are ~N(0,1); exp is safe in
fp32) and builds the denominator with a DVE accumulation over k-tiles plus an
all-ones matmul that both finishes the sum across partitions and broadcasts
it; 1/norm is exp(-ln(norm)) on ScalarE (exp and ln share one ACT table set;
the dedicated Reciprocal op is disallowed/slow).

Matmuls run in float32r (TF32-like: full rate at free-dim >= 256, measured
rel. err ~1.5e-4 for a K=2048 contraction vs fp32's 4x slowdown).
"""
import sys

sys.path.insert(0, "/opt/trn_rl_repo")

import numpy as np

import concourse.bass as bass  # noqa: F401  (engine registry import side effects)
import concourse.mybir as mybir
import concourse.tile as tile
from concourse import bacc
from concourse.bass_utils import run_bass_kernel_spmd

B, S, H, NH = 2, 2048, 2048, 16
HD = H // NH            # 128
NCORES = 8
HPC = NH // NCORES      # heads per core = 2
DLOC = HPC * HD         # 256 local head dims per core
SCALE = 1.0 / float(np.sqrt(HD))

F32 = mybir.dt.float32
F32R = mybir.dt.float32r
EXP = mybir.ActivationFunctionType.Exp
LN = mybir.ActivationFunctionType.Ln

KS = H // 128           # 16 contraction subtiles for the projections
NTT = S // 512          # 4 token tiles of 512
NKT = S // 128          # 16 key tiles of 128
NQT = S // 512          # 4 query tiles of 512
NOT_ = H // 512         # 4 output tiles of 512
NTT2 = S // 128         # 16 token tiles of 128 (output projection)


def _build():
    nc = bacc.Bacc("TRN2", target_bir_lowering=False, debug=False, num_devices=NCORES)

    xT_d = nc.dram_tensor("xT", [B, H, S], F32R, kind="ExternalInput").ap()
    ones_d = nc.dram_tensor("ones", [128, 128], F32R, kind="ExternalInput").ap()
    wq_d = nc.dram_tensor("wq", [H, DLOC], F32R, kind="ExternalInput").ap()
    wk_d = nc.dram_tensor("wk", [H, DLOC], F32R, kind="ExternalInput").ap()
    wv_d = nc.dram_tensor("wv", [H, DLOC], F32R, kind="ExternalInput").ap()
    wo_d = nc.dram_tensor("wo", [DLOC, H], F32R, kind="ExternalInput").ap()
    y_d = nc.dram_tensor("y", [B, S, H], F32, kind="ExternalOutput").ap()

    with tile.TileContext(nc) as tc:
        with tc.tile_pool(name="sb", bufs=1) as sb, \
             tc.tile_pool(name="ps", bufs=1, space="PSUM") as ps:

            ones = sb.tile([128, 128], F32R, tag="ones", bufs=1)

            # Resident weights: [128, KS, DLOC] with the contraction dim on
            # partitions; wo as [128, HPC, H].
            wq_s = sb.tile([128, KS, DLOC], F32R, tag="wq", bufs=1)
            wk_s = sb.tile([128, KS, DLOC], F32R, tag="wk", bufs=1)
            wv_s = sb.tile([128, KS, DLOC], F32R, tag="wv", bufs=1)
            wo_s = sb.tile([128, HPC, H], F32R, tag="wo", bufs=1)
            for ks in range(KS):
                nc.gpsimd.dma_start(wq_s[:, ks], wq_d[ks * 128:(ks + 1) * 128, :])
                nc.gpsimd.dma_start(wk_s[:, ks], wk_d[ks * 128:(ks + 1) * 128, :])
                nc.gpsimd.dma_start(wv_s[:, ks], wv_d[ks * 128:(ks + 1) * 128, :])
            nc.gpsimd.dma_start(ones, ones_d)
            for h in range(HPC):
                nc.gpsimd.dma_start(wo_s[:, h], wo_d[h * 128:(h + 1) * 128, :])

            for b in range(B):
                # ---- q/k/v projections (one streaming pass over xT[b]) ----
                qTb = sb.tile([128, HPC, S], F32R, tag="qTb", bufs=1)
                kTb = sb.tile([128, HPC, S], F32R, tag="kTb", bufs=1)
                vb = sb.tile([128, NKT, DLOC], F32R, tag="vb", bufs=1)
                aoTb = sb.tile([128, HPC, S], F32R, tag="aoTb", bufs=1)

                for tt in range(NTT):
                    q_ps = [ps.tile([128, 512], F32, tag="ps", bufs=8, name=f"qps{b}{tt}{m}") for m in range(2)]
                    k_ps = [ps.tile([128, 512], F32, tag="ps", bufs=8, name=f"kps{b}{tt}{m}") for m in range(2)]
                    v_ps = [ps.tile([128, 256], F32, tag="ps", bufs=8, name=f"vps{b}{tt}{m}") for m in range(4)]
                    for ks in range(KS):
                        xs = sb.tile([128, 512], F32R, tag="xs", bufs=6, name=f"xs{b}{tt}{ks}")
                        nc.sync.dma_start(
                            xs, xT_d[b, ks * 128:(ks + 1) * 128, tt * 512:(tt + 1) * 512])
                        st, sp = ks == 0, ks == KS - 1
                        for m in range(2):
                            nc.tensor.matmul(q_ps[m], wq_s[:, ks, m * 128:(m + 1) * 128],
                                             xs, start=st, stop=sp)
                        for m in range(2):
                            nc.tensor.matmul(k_ps[m], wk_s[:, ks, m * 128:(m + 1) * 128],
                                             xs, start=st, stop=sp)
                        for t4 in range(4):
                            nc.tensor.matmul(
                                v_ps[t4],
                                xs[:, t4 * 128:(t4 + 1) * 128], wv_s[:, ks],
                                start=st, stop=sp)
                    for m in range(2):
                        nc.vector.tensor_copy(qTb[:, m, tt * 512:(tt + 1) * 512], q_ps[m])
                        nc.vector.tensor_copy(kTb[:, m, tt * 512:(tt + 1) * 512], k_ps[m])
                    for t4 in range(4):
                        nc.vector.tensor_copy(vb[:, tt * 4 + t4, :], v_ps[t4])

                # ---- attention per local head ----
                for h in range(HPC):
                    unn = [ps.tile([128, 512], F32, tag="ps", bufs=8, name=f"unn{b}{h}{qt}") for qt in range(NQT)]
                    accs = [sb.tile([128, 512], F32R, tag="acc", bufs=6, name=f"acc{b}{h}{qt}") for qt in range(NQT)]
                    for kt in range(NKT):
                        st, sp = kt == 0, kt == NKT - 1
                        for qt in range(NQT):
                            sps = ps.tile([128, 512], F32, tag="ps", bufs=8, name=f"sps{b}{h}{kt}{qt}")
                            nc.tensor.matmul(sps, kTb[:, h, kt * 128:(kt + 1) * 128],
                                             qTb[:, h, qt * 512:(qt + 1) * 512],
                                             start=True, stop=True)
                            e = sb.tile([128, 512], F32R, tag="e", bufs=6, name=f"e{b}{h}{kt}{qt}")
                            nc.scalar.activation(e, sps, EXP, scale=SCALE)
                            nc.tensor.matmul(unn[qt], vb[:, kt, h * 128:(h + 1) * 128],
                                             e, start=st, stop=sp)
                            if kt == 0:
                                nc.vector.tensor_copy(accs[qt], e)
                            else:
                                nc.vector.tensor_add(accs[qt], accs[qt], e)
                    for qt in range(NQT):
                        # free the PSUM bank before the (long) normalize chain
                        ub = sb.tile([128, 512], F32, tag="ub", bufs=6, name=f"ub{b}{h}{qt}")
                        nc.vector.tensor_copy(ub, unn[qt])
                        nps = ps.tile([128, 512], F32, tag="ps", bufs=8, name=f"nps{b}{h}{qt}")
                        nc.tensor.matmul(nps, ones, accs[qt], start=True, stop=True)
                        lnn = sb.tile([128, 512], F32, tag="rc", bufs=6, name=f"ln{b}{h}{qt}")
                        nc.scalar.activation(lnn, nps, LN)
                        rc = sb.tile([128, 512], F32, tag="rc", bufs=6, name=f"rc{b}{h}{qt}")
                        nc.scalar.activation(rc, lnn, EXP, scale=-1.0)
                        nc.vector.tensor_mul(aoTb[:, h, qt * 512:(qt + 1) * 512],
                                             ub, rc)

                # ---- output projection (partial y for this core's heads) ----
                for t2 in range(NTT2):
                    y_ps = [ps.tile([128, 512], F32, tag="ps", bufs=8, name=f"yps{b}{t2}{ot}") for ot in range(NOT_)]
                    for h in range(HPC):
                        for ot in range(NOT_):
                            nc.tensor.matmul(y_ps[ot], aoTb[:, h, t2 * 128:(t2 + 1) * 128],
                                             wo_s[:, h, ot * 512:(ot + 1) * 512],
                                             start=(h == 0), stop=(h == HPC - 1))
                    for ot in range(NOT_):
                        ysb = sb.tile([128, 512], F32, tag="ysb", bufs=4, name=f"ysb{b}{t2}{ot}")
                        nc.vector.tensor_copy(ysb, y_ps[ot])
                        nc.sync.dma_start(
                            y_d[b, t2 * 128:(t2 + 1) * 128, ot * 512:(ot + 1) * 512], ysb)

    nc.compile()
    return nc


_NC = None


def _get_nc():
    global _NC
    if _NC is None:
        _NC = _build()
    return _NC


def kernel(x, Wq, Wk, Wv, Wo, bo):
    x = np.asarray(x, dtype=np.float32)
    Wq = np.asarray(Wq, dtype=np.float32)
    Wk = np.asarray(Wk, dtype=np.float32)
    Wv = np.asarray(Wv, dtype=np.float32)
    Wo = np.asarray(Wo, dtype=np.float32)
    bo = np.asarray(bo, dtype=np.float32)

    nc = _get_nc()
    xT = np.ascontiguousarray(x.transpose(0, 2, 1))
    in_maps = []
    for c in range(NCORES):
        sl = slice(c * DLOC, (c + 1) * DLOC)
        in_maps.append({
            "xT": xT,
            "ones": np.ones((128, 128), dtype=np.float32),
            "wq": np.ascontiguousarray(Wq[sl, :].T),
            "wk": np.ascontiguousarray(Wk[sl, :].T),
            "wv": np.ascontiguousarray(Wv[sl, :].T),
            "wo": np.ascontiguousarray(Wo[:, sl].T),
        })
    res = run_bass_kernel_spmd(nc, in_maps, list(range(NCORES)))
    y = np.zeros((B, S, H), dtype=np.float32)
    for c in range(NCORES):
        y += np.asarray(res.results[c]["y"])
    y += bo
    return y


# revision 7
# speedup vs baseline: 1.1276x; 1.0135x over previous
"""Memory-efficient multi-head attention block on 8 TRN2 NeuronCores.

Computes (matching torch nn.Linear conventions, W is [out, in]):
    q, k, v = x@Wq.T, x@Wk.T, x@Wv.T          [B, S, H] -> heads [B, NH, S, HD]
    out     = softmax(q k^T / sqrt(HD)) v      per head
    y       = concat_heads(out) @ Wo.T + bo

Sharding: head-parallel tensor parallelism. Each of the 8 cores owns 2 of the
16 heads: Wq/Wk/Wv are sharded on their output dim, Wo on its input dim. Each
core computes a full-shape partial y (its heads' contribution through Wo);
host sums the 8 partials and adds the bias.

Layout trick: everything on device runs transpose-free.  The host feeds
x pre-transposed as xT[b] = x[b].T ([H, S]); then
  - qT/kT (per head [HD, S]) come out of matmuls directly (weights as lhsT),
  - v     (per head [S, HD]) uses xT tiles as lhsT,
  - scores^T [k, q] = kT_tile.T @ qT (contract over HD partitions),
  - attn-out^T [HD, q] = v_tile.T @ exp(scores^T) (contract over k partitions),
  - y tiles [tok, out] = attnT_tile.T @ WoT (contract over HD partitions).
Softmax skips the max-subtraction (scaled scores are ~N(0,1); exp is safe in
fp32) and builds the denominator with a DVE accumulation over k-tiles plus an
all-ones matmul that both finishes the sum across partitions and broadcasts
it; 1/norm is exp(-ln(norm)) on ScalarE (exp and ln share one ACT table set;
the dedicated Reciprocal op is disallowed/slow).

Matmuls run in float32r (TF32-like: full rate at free-dim >= 256, measured
rel. err ~1.5e-4 for a K=2048 contraction vs fp32's 4x slowdown).

All PSUM tiles are [128, 1024] (two banks).  Where two matmul accumulation
groups share one tile they are kept bank-disjoint (a `start=True` clear is
bank-granular, so two groups in one bank corrupt each other).  Query tiles are
processed in pairs so exp / row-sum accumulation run 1024 wide, amortizing the
per-instruction overhead of ScalarE/VectorE.
"""
import sys

sys.path.insert(0, "/opt/trn_rl_repo")

import numpy as np

import concourse.bass as bass  # noqa: F401  (engine registry import side effects)
import concourse.mybir as mybir
import concourse.tile as tile
from concourse import bacc
from concourse.bass_utils import run_bass_kernel_spmd

B, S, H, NH = 2, 2048, 2048, 16
HD = H // NH            # 128
NCORES = 8
HPC = NH // NCORES      # heads per core = 2
DLOC = HPC * HD         # 256 local head dims per core
SCALE = 1.0 / float(np.sqrt(HD))

F32 = mybir.dt.float32
F32R = mybir.dt.float32r
EXP = mybir.ActivationFunctionType.Exp
LN = mybir.ActivationFunctionType.Ln

KS = H // 128           # 16 contraction subtiles for the projections
NTT = S // 512          # 4 token tiles of 512 (projection rhs width)
NKT = S // 128          # 16 key tiles of 128
NPR = S // 1024         # 2 query-pair tiles of 1024
NTT2 = S // 128         # 16 token tiles of 128 (output projection)
NOT2 = H // 1024        # 2 output tiles of 1024 (output projection)


def _build():
    nc = bacc.Bacc("TRN2", target_bir_lowering=False, debug=False, num_devices=NCORES)

    xT_d = nc.dram_tensor("xT", [B, H, S], F32R, kind="ExternalInput").ap()
    ones_d = nc.dram_tensor("ones", [128, 128], F32R, kind="ExternalInput").ap()
    wq_d = nc.dram_tensor("wq", [H, DLOC], F32R, kind="ExternalInput").ap()
    wk_d = nc.dram_tensor("wk", [H, DLOC], F32R, kind="ExternalInput").ap()
    wv_d = nc.dram_tensor("wv", [H, DLOC], F32R, kind="ExternalInput").ap()
    wo_d = nc.dram_tensor("wo", [DLOC, H], F32R, kind="ExternalInput").ap()
    y_d = nc.dram_tensor("y", [B, S, H], F32, kind="ExternalOutput").ap()

    with tile.TileContext(nc) as tc:
        with tc.tile_pool(name="sb", bufs=1) as sb, \
             tc.tile_pool(name="ps", bufs=1, space="PSUM") as ps:

            def p2(name):
                return ps.tile([128, 1024], F32, tag="p2", bufs=4, name=name)

            ones = sb.tile([128, 128], F32R, tag="ones", bufs=1)

            wq_s = sb.tile([128, KS, DLOC], F32R, tag="wq", bufs=1)
            wk_s = sb.tile([128, KS, DLOC], F32R, tag="wk", bufs=1)
            wv_s = sb.tile([128, KS, DLOC], F32R, tag="wv", bufs=1)
            wo_s = sb.tile([128, HPC, H], F32R, tag="wo", bufs=1)
            for ks in range(KS):
                nc.gpsimd.dma_start(wq_s[:, ks], wq_d[ks * 128:(ks + 1) * 128, :])
                nc.gpsimd.dma_start(wk_s[:, ks], wk_d[ks * 128:(ks + 1) * 128, :])
                nc.gpsimd.dma_start(wv_s[:, ks], wv_d[ks * 128:(ks + 1) * 128, :])
            nc.gpsimd.dma_start(ones, ones_d)
            for h in range(HPC):
                nc.gpsimd.dma_start(wo_s[:, h], wo_d[h * 128:(h + 1) * 128, :])

            qTb = [None] * B
            kTb = [None] * B
            vb = [None] * B
            aoTb = [None] * B

            def proj(b):
                # q/k/v projections: one streaming pass over xT[b].
                qTb[b] = sb.tile([128, HPC, S], F32R, tag="qTb", bufs=1, name=f"qTb{b}")
                kTb[b] = sb.tile([128, HPC, S], F32R, tag="kTb", bufs=1, name=f"kTb{b}")
                vb[b] = sb.tile([128, NKT, DLOC], F32R, tag="vb", bufs=1, name=f"vb{b}")
                aoTb[b] = sb.tile([128, HPC, S], F32R, tag="aoTb", bufs=1, name=f"aoTb{b}")
                for tt in range(NTT):
                    # two heads' q (or k) share one 2-bank psum tile, one bank
                    # per head; v gets one bank per 128-token subtile (only
                    # 256 of each bank's 512 lanes are used).
                    q_ps = p2(f"qps{b}{tt}")
                    k_ps = p2(f"kps{b}{tt}")
                    v_ps = [p2(f"vps{b}{tt}{i}") for i in range(2)]
                    for ks in range(KS):
                        xs = sb.tile([128, 512], F32R, tag="xs", bufs=6, name=f"xs{b}{tt}{ks}")
                        nc.sync.dma_start(
                            xs, xT_d[b, ks * 128:(ks + 1) * 128, tt * 512:(tt + 1) * 512])
                        st, sp = ks == 0, ks == KS - 1
                        for m in range(2):
                            nc.tensor.matmul(q_ps[:, m * 512:(m + 1) * 512],
                                             wq_s[:, ks, m * 128:(m + 1) * 128],
                                             xs, start=st, stop=sp)
                        for m in range(2):
                            nc.tensor.matmul(k_ps[:, m * 512:(m + 1) * 512],
                                             wk_s[:, ks, m * 128:(m + 1) * 128],
                                             xs, start=st, stop=sp)
                        for t4 in range(4):
                            nc.tensor.matmul(
                                v_ps[t4 // 2][:, (t4 % 2) * 512:(t4 % 2) * 512 + 256],
                                xs[:, t4 * 128:(t4 + 1) * 128], wv_s[:, ks],
                                start=st, stop=sp)
                    for m in range(2):
                        nc.vector.tensor_copy(qTb[b][:, m, tt * 512:(tt + 1) * 512],
                                              q_ps[:, m * 512:(m + 1) * 512])
                        nc.vector.tensor_copy(kTb[b][:, m, tt * 512:(tt + 1) * 512],
                                              k_ps[:, m * 512:(m + 1) * 512])
                    for t4 in range(4):
                        nc.vector.tensor_copy(
                            vb[b][:, tt * 4 + t4, :],
                            v_ps[t4 // 2][:, (t4 % 2) * 512:(t4 % 2) * 512 + 256])

            def attn(b, h):
                for pr in range(NPR):
                    unn2 = p2(f"unn{b}{h}{pr}")
                    acc2 = sb.tile([128, 1024], F32R, tag="acc", bufs=4, name=f"acc{b}{h}{pr}")
                    q0 = pr * 1024
                    for kt in range(NKT):
                        st, sp = kt == 0, kt == NKT - 1
                        s2 = p2(f"sps{b}{h}{pr}{kt}")
                        for i in range(2):
                            nc.tensor.matmul(
                                s2[:, i * 512:(i + 1) * 512],
                                kTb[b][:, h, kt * 128:(kt + 1) * 128],
                                qTb[b][:, h, q0 + i * 512:q0 + (i + 1) * 512],
                                start=True, stop=True)
                        e2 = sb.tile([128, 1024], F32R, tag="e", bufs=4, name=f"e{b}{h}{pr}{kt}")
                        nc.scalar.activation(e2, s2, EXP, scale=SCALE)
                        for i in range(2):
                            nc.tensor.matmul(
                                unn2[:, i * 512:(i + 1) * 512],
                                vb[b][:, kt, h * 128:(h + 1) * 128],
                                e2[:, i * 512:(i + 1) * 512],
                                start=st, stop=sp)
                        if kt == 0:
                            nc.vector.tensor_copy(acc2, e2)
                        else:
                            nc.vector.tensor_add(acc2, acc2, e2)
                    # free the two unn banks before the (long) normalize chain
                    ub2 = sb.tile([128, 1024], F32, tag="ub", bufs=2, name=f"ub{b}{h}{pr}")
                    nc.vector.tensor_copy(ub2, unn2)
                    nps2 = p2(f"nps{b}{h}{pr}")
                    for i in range(2):
                        nc.tensor.matmul(nps2[:, i * 512:(i + 1) * 512], ones,
                                         acc2[:, i * 512:(i + 1) * 512],
                                         start=True, stop=True)
                    lnn2 = sb.tile([128, 1024], F32, tag="rc", bufs=3, name=f"ln{b}{h}{pr}")
                    nc.scalar.activation(lnn2, nps2, LN)
                    rc2 = sb.tile([128, 1024], F32, tag="rc", bufs=3, name=f"rc{b}{h}{pr}")
                    nc.scalar.activation(rc2, lnn2, EXP, scale=-1.0)
                    nc.vector.tensor_mul(aoTb[b][:, h, q0:q0 + 1024], ub2, rc2)

            def outproj(b):
                for t2 in range(NTT2):
                    for o2 in range(NOT2):
                        y2 = p2(f"yps{b}{t2}{o2}")
                        for h in range(HPC):
                            for i in range(2):
                                nc.tensor.matmul(
                                    y2[:, i * 512:(i + 1) * 512],
                                    aoTb[b][:, h, t2 * 128:(t2 + 1) * 128],
                                    wo_s[:, h, o2 * 1024 + i * 512:o2 * 1024 + (i + 1) * 512],
                                    start=(h == 0), stop=(h == HPC - 1))
                        ysb = sb.tile([128, 1024], F32, tag="ysb", bufs=3, name=f"ysb{b}{t2}{o2}")
                        nc.any.tensor_copy(ysb, y2)
                        nc.sync.dma_start(
                            y_d[b, t2 * 128:(t2 + 1) * 128, o2 * 1024:(o2 + 1) * 1024], ysb)

            proj(0)
            attn(0, 0)
            attn(0, 1)
            proj(1)
            attn(1, 0)
            outproj(0)   # PE-heavy; fills batch-1 attention's ACT-bound gaps
            attn(1, 1)
            outproj(1)

    nc.compile()
    return nc


_NC = None


def _get_nc():
    global _NC
    if _NC is None:
        _NC = _build()
    return _NC


def kernel(x, Wq, Wk, Wv, Wo, bo):
    x = np.asarray(x, dtype=np.float32)
    Wq = np.asarray(Wq, dtype=np.float32)
    Wk = np.asarray(Wk, dtype=np.float32)
    Wv = np.asarray(Wv, dtype=np.float32)
    Wo = np.asarray(Wo, dtype=np.float32)
    bo = np.asarray(bo, dtype=np.float32)

    nc = _get_nc()
    xT = np.ascontiguousarray(x.transpose(0, 2, 1))
    in_maps = []
    for c in range(NCORES):
        sl = slice(c * DLOC, (c + 1) * DLOC)
        in_maps.append({
            "xT": xT,
            "ones": np.ones((128, 128), dtype=np.float32),
            "wq": np.ascontiguousarray(Wq[sl, :].T),
            "wk": np.ascontiguousarray(Wk[sl, :].T),
            "wv": np.ascontiguousarray(Wv[sl, :].T),
            "wo": np.ascontiguousarray(Wo[:, sl].T),
        })
    res = run_bass_kernel_spmd(nc, in_maps, list(range(NCORES)))
    y = np.zeros((B, S, H), dtype=np.float32)
    for c in range(NCORES):
        y += np.asarray(res.results[c]["y"])
    y += bo
    return y
